# revision 10
# baseline (speedup 1.0000x reference)
"""GeomGCN (2-layer relational GCN) distributed Bass kernel for 8 TRN2 NeuronCores.

Strategy (source-sharded, graph-parallel, bf16 hot path):
  - Nodes split into 8 slices of NLOC (multiple of 128); core k owns slice k
    and all edges whose source `col` lies in it.
  - Message tables are (node, rel)-combined: row index = node*4 + rel, so a
    node's 4 relation messages are 1KB-contiguous and each table is written
    with ONE fat DMA.  Both layers share one gather-index table
    (idx = col_local*4 + rel), resident in SBUF.
  - Gather slots use grouped continuous packing: per group of G dest tiles,
    each core packs its edges continuously into shared 128-slot blocks; the
    static schedule is a list of (block, tile) chunks whose one-hot S is
    built per chunk from a chunk-indexed dloc column (sentinel -1 for
    foreign/dummy slots).  This cuts dummy-slot gather waste from ~50% to
    ~6%.
  - Layer-1 aggregation is TRANSPOSED on TensorE (lhsT = gathered messages,
    rhs = S) so partials come out [H, node]; each dest tile is then
    immediately multiplied by W2cat on-core ((D t1)@W2 = D (t1@W2)), so the
    ReduceScatter moves 64-wide y2 partials and the layer-2 dense phase
    disappears.  dinv^2 scaling + b1-path bias are applied post-RS while
    assembling the layer-2 table.
  - Layer-2 aggregation is node-major (lhsT = S), 16-wide; after a second
    ReduceScatter a fused log_softmax (2 activation table loads total)
    emits a transposed [128, MC*16] output that the host un-transposes.
  Host work: index prep, degree/dinv, pre-scaling x by dinv, bf16 packing.
"""
import math
import os
import numpy as np
import ml_dtypes

import concourse.bass as bass
import concourse.tile as tile
from concourse import bacc, mybir
from concourse.bass_utils import run_bass_kernel_spmd

F32 = mybir.dt.float32
BF16 = mybir.dt.bfloat16
I16 = mybir.dt.int16
AF = mybir.ActivationFunctionType
ALU = mybir.AluOpType
BF_NP = ml_dtypes.bfloat16


class Cfg:
    def __init__(self, N, E, F, H, C, R, ncores=8, B=8, G=8, J=8):
        self.N, self.E, self.F, self.H, self.C, self.R = N, E, F, H, C, R
        self.ncores = ncores
        self.P = 128
        # node slice per core, multiple of 128 so dest tiles align to cores
        self.NLOC = math.ceil(N / ncores / 128) * 128
        self.MC = self.NLOC // 128                   # dest tiles per core
        self.MPAD = self.NLOC                        # padded nodes per core
        self.N_PAD = self.NLOC * ncores
        self.NT = self.N_PAD // 128                  # total dest tiles
        self.DUMMY = self.MPAD * R                   # zero row in tables
        self.B = B                                   # blocks per gather batch
        self.G = G                                   # tiles per packing group
        self.KC = F // 128                           # k-chunks layer-1 dense
        assert F % 128 == 0 and H == 128
        assert self.MPAD * R + 128 < 32768, "int16 gather index overflow"


CFG = Cfg(N=50000, E=800000, F=256, H=128, C=16, R=4,
          B=int(os.environ.get("KB", "8")), G=int(os.environ.get("KG", "8")))


def _bf(a):
    return np.ascontiguousarray(np.asarray(a, dtype=np.float32).astype(BF_NP))


# ----------------------------------------------------------------- host side
def preprocess(cfg, x, edge_index, edge_relation, W1, b1, W2, b2):
    N, ncores, NLOC, NT, R, MC, G = (cfg.N, cfg.ncores, cfg.NLOC, cfg.NT,
                                     cfg.R, cfg.MC, cfg.G)
    row = np.asarray(edge_index[0], dtype=np.int64)
    col = np.asarray(edge_index[1], dtype=np.int64)
    rel = np.asarray(edge_relation, dtype=np.int64)
    x = np.asarray(x, dtype=np.float32)
    W1 = np.asarray(W1, dtype=np.float32)
    b1 = np.asarray(b1, dtype=np.float32)
    W2 = np.asarray(W2, dtype=np.float32)
    b2 = np.asarray(b2, dtype=np.float32)

    deg = np.bincount(row, minlength=N).astype(np.float32)
    dinv = np.where(deg > 0, 1.0 / np.sqrt(np.maximum(deg, 1.0)),
                    0.0).astype(np.float32)

    # per-core edge sets (by source/col ownership), sorted by dest row
    per_core = []
    counts = np.zeros((ncores, NT), dtype=np.int64)
    for k in range(ncores):
        m = (col // NLOC) == k
        er, ec, eg = row[m], col[m] - k * NLOC, rel[m]
        o = np.argsort(er, kind="stable")
        er, ec, eg = er[o], ec[o], eg[o]
        counts[k] = np.bincount(er // 128, minlength=NT)
        per_core.append((er, ec, eg))

    # ---- grouped continuous packing (shared static schedule)
    # groups of up to G tiles, never crossing a dest-core boundary
    groups = []
    for dk in range(ncores):
        t0 = dk * MC
        for g0 in range(0, MC, G):
            groups.append((t0 + g0, t0 + min(MC, g0 + G)))
    gi_of_tile = np.zeros(NT, dtype=np.int64)
    for gi, (tlo, thi) in enumerate(groups):
        gi_of_tile[tlo:thi] = gi
    csum = [np.concatenate([[0], np.cumsum(counts[k])]) for k in range(ncores)]
    tiles_chunks = [[] for _ in range(NT)]   # per tile: list of block ids
    SB = 0
    core_gbase = np.zeros((ncores, len(groups)), dtype=np.int64)
    for gi, (tlo, thi) in enumerate(groups):
        nblk = 1
        for k in range(ncores):
            core_gbase[k, gi] = SB * 128
            sgk = int(csum[k][thi] - csum[k][tlo])
            nblk = max(nblk, math.ceil(sgk / 128))
        for t in range(tlo, thi):
            lo, hi = None, None
            for k in range(ncores):
                p0 = int(csum[k][t] - csum[k][tlo])
                p1 = int(csum[k][t + 1] - csum[k][tlo])
                if p1 > p0:
                    l, h = p0 // 128, (p1 - 1) // 128
                    lo = l if lo is None else min(lo, l)
                    hi = h if hi is None else max(hi, h)
            if lo is None:
                lo = hi = 0
            tiles_chunks[t] = [SB + b for b in range(lo, hi + 1)]
        SB += nblk
    NBAT = math.ceil(SB / cfg.B)
    SBpad = NBAT * cfg.B
    # chunk order = tile-major; chunk index per (tile, block)
    chunk_of = {}
    CH = 0
    for t in range(NT):
        for b in tiles_chunks[t]:
            chunk_of[(t, b)] = CH
            CH += 1

    # shared weight packs
    w1cat = np.zeros((cfg.KC, 128, R * cfg.H), dtype=np.float32)
    for r in range(R):
        for kc in range(cfg.KC):
            w1cat[kc, :, r * cfg.H:(r + 1) * cfg.H] = \
                W1[r * cfg.F + kc * 128: r * cfg.F + (kc + 1) * 128, :]
    w1cat = _bf(w1cat.reshape(cfg.KC * 128, R * cfg.H))
    w2cat = np.zeros((cfg.H, R * cfg.C), dtype=np.float32)
    for r in range(R):
        w2cat[:, r * cfg.C:(r + 1) * cfg.C] = W2[r * cfg.H:(r + 1) * cfg.H, :]
    w2cat = _bf(w2cat)
    crow1 = np.concatenate([b1 @ W2[r * cfg.H:(r + 1) * cfg.H, :]
                            for r in range(R)])
    crow = np.broadcast_to(crow1.astype(np.float32), (128, R * cfg.C)).copy()
    iota2 = _bf(np.broadcast_to(np.arange(128, dtype=np.float32), (128, 128)))
    b2r = np.broadcast_to(b2, (128, cfg.C)).astype(np.float32).copy()

    in_maps = []
    for k in range(ncores):
        er, ec, eg = per_core[k]
        t = er // 128
        egi = gi_of_tile[t]
        # position within group = running index of edge within its group
        first_of_group = np.concatenate(
            [[0], np.cumsum(np.bincount(egi, minlength=len(groups)))])[:-1]
        pos_in_group = np.arange(len(er)) - first_of_group[egi]
        slots = core_gbase[k][egi] + pos_in_group

        gidx = np.full(SBpad * 128, cfg.DUMMY, dtype=np.int16)
        gidx[slots] = (ec * R + eg).astype(np.int16)
        # chunk-indexed dloc: for chunk (t, b): slot p of block b ->
        #   er%128 if the edge belongs to tile t else -1
        dloc = np.full((CH, 128), -1.0, dtype=np.float32)
        eb = slots // 128
        ep = slots % 128
        eci = np.fromiter((chunk_of[(int(tt), int(bb))]
                           for tt, bb in zip(t, eb)),
                          dtype=np.int64, count=len(er))
        dloc[eci, ep] = (er % 128).astype(np.float32)

        # wrapped-16 int16 index layout, replicated to 8 Q7 groups, blocked
        # into one [128, NBAT*B*8] resident SBUF tile
        gg = gidx.reshape(NBAT, cfg.B * 8, 16)
        w = np.transpose(gg, (0, 2, 1))
        gidx_w = np.broadcast_to(
            w[:, None, :, :], (NBAT, 8, 16, cfg.B * 8)
        ).reshape(NBAT, 128, cfg.B * 8)
        gidx_sb = np.ascontiguousarray(
            np.transpose(gidx_w, (1, 0, 2)).reshape(128, NBAT * cfg.B * 8))
        dloc_sb = np.ascontiguousarray(dloc.T)       # [128, CH]

        lo = k * NLOC
        hi = min(N, lo + NLOC)
        nk = hi - lo
        uk = np.zeros((cfg.MPAD, cfg.F), dtype=np.float32)
        uk[:nk] = x[lo:hi] * dinv[lo:hi, None]
        dk = np.zeros(cfg.MPAD, dtype=np.float32)
        dk[:nk] = dinv[lo:hi]

        RC = R * cfg.C
        dinvcr = np.ascontiguousarray(
            np.repeat(dk.reshape(MC, 128), cfg.C, axis=1)
            .reshape(MC, 128, cfg.C).transpose(1, 0, 2)
            .reshape(128, MC * cfg.C))                            # [128, MC*C]
        dinvrc = np.ascontiguousarray(
            np.repeat(dk.reshape(MC, 128), RC, axis=1)
            .reshape(MC, 128, RC).transpose(1, 0, 2)
            .reshape(128, MC * RC))                               # [128, MC*RC]

        in_maps.append({
            "uT": _bf(uk.T),                     # [F, MPAD]
            "w1cat": w1cat,                      # [KC*128, R*H]
            "w2cat": w2cat,                      # [H, R*C]
            "crow": crow,                        # [128, R*C] f32
            "iota2": iota2,                      # [128, 128]
            "b2r": b2r,                          # [128, C] f32
            "gidx": gidx_sb,                     # [128, NBAT*B*8] i16
            "dloc": dloc_sb,                     # [128, CH] f32
            "dinvrc": dinvrc.astype(np.float32),  # [128, MC*RC] f32
            "dinvcr": dinvcr.astype(np.float32),  # [128, MC*C] f32
        })
    sched = tuple(tuple(tc_) for tc_ in tiles_chunks)
    return in_maps, sched, SBpad


# --------------------------------------------------------------- device side
def build_program(cfg, sched, SBpad):
    R, H, C = cfg.R, cfg.H, cfg.C
    NBAT = SBpad // cfg.B
    CH = sum(len(s) for s in sched)
    nc = bacc.Bacc("TRN2", target_bir_lowering=False, debug=False,
                   num_devices=cfg.ncores)

    uT = nc.dram_tensor("uT", [cfg.F, cfg.MPAD], BF16, kind="ExternalInput").ap()
    w1cat = nc.dram_tensor("w1cat", [cfg.KC * 128, R * H], BF16,
                           kind="ExternalInput").ap()
    w2cat = nc.dram_tensor("w2cat", [H, R * C], BF16, kind="ExternalInput").ap()
    crow = nc.dram_tensor("crow", [128, R * C], F32, kind="ExternalInput").ap()
    iota2 = nc.dram_tensor("iota2", [128, 128], BF16, kind="ExternalInput").ap()
    b2r = nc.dram_tensor("b2r", [128, C], F32, kind="ExternalInput").ap()
    gidx = nc.dram_tensor("gidx", [128, NBAT * cfg.B * 8], I16,
                          kind="ExternalInput").ap()
    dloc = nc.dram_tensor("dloc", [128, CH], F32, kind="ExternalInput").ap()
    dinvrc = nc.dram_tensor("dinvrc", [128, cfg.MC * R * C], F32,
                            kind="ExternalInput").ap()
    dinvcr = nc.dram_tensor("dinvcr", [128, cfg.MC * C], F32,
                            kind="ExternalInput").ap()
    outT = nc.dram_tensor("outT", [128, cfg.MC * C], F32,
                          kind="ExternalOutput").ap()

    with tile.TileContext(nc) as tc:
        _build(tc, cfg, sched, SBpad, uT, w1cat, w2cat, crow, iota2, b2r,
               gidx, dloc, dinvrc, dinvcr, outT)
    nc.compile()
    return nc


def _build(tc, cfg, sched, SBpad, uT, w1cat, w2cat, crow, iota2, b2r,
           gidx, dloc, dinvrc, dinvcr, outT):
    nc = tc.nc
    R, H, C, B, MC, NT, KC = (cfg.R, cfg.H, cfg.C, cfg.B, cfg.MC, cfg.NT,
                              cfg.KC)
    NBAT = SBpad // B
    NCORES = cfg.ncores
    TROWS = cfg.MPAD * R
    CH = sum(len(s) for s in sched)
    RC = R * C
    with tc.tile_pool(name="const", bufs=1) as cpool, \
         tc.tile_pool(name="big", bufs=1) as bigp, \
         tc.tile_pool(name="stg", bufs=2) as stgp, \
         tc.tile_pool(name="gY", bufs=3) as gpool, \
         tc.tile_pool(name="s3", bufs=12) as spool, \
         tc.tile_pool(name="ev", bufs=4) as evp, \
         tc.tile_pool(name="psA", bufs=2, space="PSUM") as ppa, \
         tc.tile_pool(name="psB", bufs=6, space="PSUM") as ppb, \
         tc.tile_pool(name="dram", bufs=1, space="DRAM") as dram:

        # ---------- resident inputs (dense-phase operands first)
        uT_t = bigp.tile([128, KC, cfg.MPAD], BF16)
        nc.sync.dma_start(
            out=uT_t[:],
            in_=uT.rearrange("(kc p) n -> p kc n", kc=KC, p=128))
        w1_t = cpool.tile([128, KC, R * H], BF16)
        nc.sync.dma_start(
            out=w1_t[:],
            in_=w1cat.rearrange("(kc p) n -> p kc n", kc=KC, p=128))
        w2_t = cpool.tile([128, RC], BF16)
        nc.sync.dma_start(out=w2_t[:], in_=w2cat[:, :])
        iota_t = cpool.tile([128, 128], BF16)
        nc.sync.dma_start(out=iota_t[:], in_=iota2[:, :])
        gidx_t = bigp.tile([128, NBAT * B * 8], I16)
        nc.sync.dma_start(out=gidx_t[:], in_=gidx[:, :])
        dloc_t = bigp.tile([128, CH], F32)
        nc.sync.dma_start(out=dloc_t[:], in_=dloc[:, :])
        crow_t = cpool.tile([128, RC], F32)
        nc.sync.dma_start(out=crow_t[:], in_=crow[:, :])
        b2_t = cpool.tile([128, C], F32)
        nc.sync.dma_start(out=b2_t[:], in_=b2r[:, :])
        dinvrc_t = bigp.tile([128, MC * RC], F32)
        nc.sync.dma_start(out=dinvrc_t[:], in_=dinvrc[:, :])
        zrow = cpool.tile([128, 128], BF16)
        nc.vector.memset(zrow[:], 0.0)

        # shared table staging [128, MC*R*H] (y1: all cols; y2: :C per block)
        stage = bigp.tile([128, MC * R * H], BF16)

        # DRAM tensors
        y1_dram = dram.tile([TROWS + 128, H], BF16)
        y2_dram = dram.tile([TROWS + 128, H], BF16)
        y2_part = dram.tile([NCORES * 128, MC * RC], BF16)
        y2_red = dram.tile([128, MC * RC], BF16)
        t2_part = dram.tile([NCORES * 128, MC * C], BF16)
        t2_red = dram.tile([128, MC * C], BF16)

        # ---------- layer-1 dense: stage[p, mc*512 + r*H + h]
        for mc in range(MC):
            ps = ppa.tile([128, R * H], F32, tag="psA")
            for kc in range(KC):
                nc.tensor.matmul(
                    out=ps[:],
                    lhsT=uT_t[:, kc, mc * 128:(mc + 1) * 128],
                    rhs=w1_t[:, kc, :],
                    start=(kc == 0), stop=(kc == KC - 1))
            if mc % 2 == 0:
                nc.scalar.copy(out=stage[:, mc * R * H:(mc + 1) * R * H],
                               in_=ps[:])
            else:
                nc.vector.tensor_scalar(
                    out=stage[:, mc * R * H:(mc + 1) * R * H],
                    in0=ps[:], scalar1=1.0, scalar2=None, op0=ALU.mult)
        nc.sync.dma_start(
            out=y1_dram[0:TROWS, :].rearrange(
                "(mc p r) h -> p mc (r h)", mc=MC, p=128, r=R),
            in_=stage[:])
        nc.sync.dma_start(out=y1_dram[TROWS:TROWS + 128, :], in_=zrow[:])

        LIMIT = int(os.environ.get("KLIMIT", "6"))
        if LIMIT < 2:
            return

        # ---------- shared gather + one-hot segment-sum pass
        def agg_pass(table_ap, width, part_dram, fuse_w2, tagsfx):
            batches = {}

            def batch(b):
                if b not in batches:
                    g = gpool.tile([128, B, H], BF16, tag="g")
                    nc.gpsimd.dma_gather(
                        out_ap=g[:], in_ap=table_ap,
                        idxs_ap=gidx_t[:, b * B * 8:(b + 1) * B * 8],
                        num_idxs=B * 128, num_idxs_reg=B * 128,
                        elem_size=H)
                    batches[b] = g
                return batches[b]

            TB = 4
            w0 = 128 if fuse_w2 else width
            ci = 0
            st = None
            ps = None
            for t in range(NT):
                k, j = t // MC, t % MC
                if j == 0:
                    st = stgp.tile([128, MC * width], BF16, tag=f"st{tagsfx}")
                jg = j % TB
                if jg == 0:
                    ntb = min(TB, MC - j)
                    ps = ppb.tile([128, TB * w0], F32, tag="ps")
                blocks = sched[t]
                for jj, b in enumerate(blocks):
                    g = batch(b // B)
                    s3 = spool.tile([128, 128], BF16, tag="s3")
                    nc.vector.tensor_scalar(
                        out=s3[:], in0=iota_t[:],
                        scalar1=dloc_t[:, ci:ci + 1],
                        scalar2=None, op0=ALU.is_equal)
                    if fuse_w2:
                        nc.tensor.matmul(
                            out=ps[:, jg * w0:(jg + 1) * w0],
                            lhsT=g[:, b % B, :], rhs=s3[:],
                            start=(jj == 0), stop=(jj == len(blocks) - 1))
                    else:
                        nc.tensor.matmul(
                            out=ps[:, jg * w0:(jg + 1) * w0],
                            lhsT=s3[:], rhs=g[:, b % B, :width],
                            start=(jj == 0), stop=(jj == len(blocks) - 1))
                    ci += 1
                if jg == ntb - 1:
                    j0 = j - ntb + 1
                    if fuse_w2:
                        # t1T tiles [h, node] -> (t1 @ W2cat) [node, RC]
                        tb = evp.tile([128, TB * 128], BF16, tag="t1T")
                        nc.scalar.copy(out=tb[:, :ntb * 128],
                                       in_=ps[:, :ntb * 128])
                        ps2 = ppa.tile([128, TB * RC], F32, tag="psA")
                        for q in range(ntb):
                            nc.tensor.matmul(
                                out=ps2[:, q * RC:(q + 1) * RC],
                                lhsT=tb[:, q * 128:(q + 1) * 128],
                                rhs=w2_t[:], start=True, stop=True)
                        nc.scalar.copy(
                            out=st[:, j0 * width:(j + 1) * width],
                            in_=ps2[:, :ntb * RC])
                    else:
                        nc.scalar.copy(
                            out=st[:, j0 * width:(j + 1) * width],
                            in_=ps[:, :ntb * width])
                if j == MC - 1:
                    nc.sync.dma_start(
                        out=part_dram[k * 128:(k + 1) * 128, :], in_=st[:])

        # ---------- layer-1 aggregation fused with W2 + reduce-scatter
        agg_pass(y1_dram[:, :], RC, y2_part, True, "1")
        if LIMIT < 3:
            return
        nc.gpsimd.collective_compute(
            "ReduceScatter", ALU.add,
            replica_groups=[list(range(NCORES))],
            ins=[y2_part.opt()], outs=[y2_red.opt()])
        if LIMIT < 4:
            return

        # ---------- layer-2 table: y2 = dinv * (dinv * red + crow)
        y2r_t = bigp.tile([128, MC * RC], BF16)
        nc.sync.dma_start(out=y2r_t[:], in_=y2_red[:, :])
        scal = bigp.tile([128, MC * RC], F32)
        nc.vector.tensor_tensor(out=scal[:], in0=y2r_t[:], in1=dinvrc_t[:],
                                op=ALU.mult)
        nc.vector.tensor_tensor(
            out=scal[:].rearrange("p (m rc) -> p m rc", rc=RC),
            in0=scal[:].rearrange("p (m rc) -> p m rc", rc=RC),
            in1=crow_t[:].unsqueeze(1).to_broadcast([128, MC, RC]),
            op=ALU.add)
        nc.vector.tensor_tensor(out=scal[:], in0=scal[:], in1=dinvrc_t[:],
                                op=ALU.mult)
        nc.vector.tensor_scalar(
            out=stage[:].rearrange("p (m r h) -> p m r h", r=R, h=H)
                [:, :, :, 0:C],
            in0=scal[:].rearrange("p (m r c) -> p m r c", r=R, c=C),
            scalar1=1.0, scalar2=None, op0=ALU.mult)
        nc.sync.dma_start(
            out=y2_dram[0:TROWS, :].rearrange(
                "(mc p r) h -> p mc (r h)", mc=MC, p=128, r=R),
            in_=stage[:])
        nc.sync.dma_start(out=y2_dram[TROWS:TROWS + 128, :], in_=zrow[:])
        if LIMIT < 5:
            return

        # ---------- layer-2 aggregation (node-major) + reduce-scatter
        agg_pass(y2_dram[:, :], C, t2_part, False, "2")
        nc.gpsimd.collective_compute(
            "ReduceScatter", ALU.add,
            replica_groups=[list(range(NCORES))],
            ins=[t2_part.opt()], outs=[t2_red.opt()])
        if LIMIT < 6:
            return

        # ---------- final: h2 = dinv*t2 + b2 ; log_softmax over C
        dinvcr_t = bigp.tile([128, MC * C], F32)
        nc.sync.dma_start(out=dinvcr_t[:], in_=dinvcr[:, :])
        t2r_t = bigp.tile([128, MC * C], BF16)
        nc.sync.dma_start(out=t2r_t[:], in_=t2_red[:, :])
        h2 = bigp.tile([128, MC, C], F32)
        nc.vector.tensor_tensor(
            out=h2[:], in0=t2r_t[:].rearrange("p (m c) -> p m c", c=C),
            in1=dinvcr_t[:].rearrange("p (m c) -> p m c", c=C), op=ALU.mult)
        nc.vector.tensor_tensor(
            out=h2[:], in0=h2[:],
            in1=b2_t[:].unsqueeze(1).to_broadcast([128, MC, C]), op=ALU.add)
        negmx = bigp.tile([128, MC, 1], F32)
        nc.vector.tensor_reduce(out=negmx[:], in_=h2[:],
                                axis=mybir.AxisListType.X,
                                op=ALU.max, negate=True)
        nc.vector.tensor_tensor(
            out=h2[:], in0=h2[:], in1=negmx[:].to_broadcast([128, MC, C]),
            op=ALU.add)
        ex = bigp.tile([128, MC, C], F32)
        nc.scalar.activation(out=ex[:], in_=h2[:], func=AF.Exp)
        ssum = bigp.tile([128, MC, 1], F32)
        nc.vector.tensor_reduce(out=ssum[:], in_=ex[:],
                                axis=mybir.AxisListType.X, op=ALU.add)
        lg = bigp.tile([128, MC, 1], F32)
        nc.scalar.activation(out=lg[:], in_=ssum[:], func=AF.Ln)
        nc.vector.tensor_tensor(
            out=h2[:], in0=h2[:], in1=lg[:].to_broadcast([128, MC, C]),
            op=ALU.subtract)
        nc.sync.dma_start(
            out=outT[:, :], in_=h2[:].rearrange("p m c -> p (m c)"))


# ------------------------------------------------------------------ runtime
_PROGRAM_CACHE = {}


def run(cfg, inputs):
    in_maps, sched, SBpad = preprocess(cfg, **inputs)
    key = (cfg.N, cfg.E, sched, SBpad)
    if key not in _PROGRAM_CACHE:
        _PROGRAM_CACHE[key] = build_program(cfg, sched, SBpad)
    nc = _PROGRAM_CACHE[key]
    res = None
    for attempt in range(3):
        try:
            res = run_bass_kernel_spmd(nc, in_maps,
                                       core_ids=list(range(cfg.ncores)))
            break
        except Exception:
            if attempt == 2:
                raise
    outs = []
    for k in range(cfg.ncores):
        oT = res.results[k]["outT"]                       # [128, MC*C]
        o = oT.reshape(128, cfg.MC, cfg.C).transpose(1, 0, 2).reshape(
            cfg.NLOC, cfg.C)
        outs.append(o)
    full = np.concatenate(outs, axis=0)[:cfg.N]
    return np.ascontiguousarray(full.astype(np.float32))


def kernel(x, edge_index, edge_relation, W1, b1, W2, b2):
    return run(CFG, dict(x=x, edge_index=edge_index,
                         edge_relation=edge_relation,
                         W1=W1, b1=b1, W2=W2, b2=b2))


# revision 11
# speedup vs baseline: 1.0125x; 1.0125x over previous
"""GeomGCN (2-layer relational GCN) distributed Bass kernel for 8 TRN2 NeuronCores.

Strategy (source-sharded, graph-parallel, bf16 hot path):
  - Nodes split into 8 slices of NLOC (multiple of 128); core k owns slice k
    and all edges whose source `col` lies in it.
  - Message tables are (node, rel)-combined: row index = node*4 + rel, so a
    node's 4 relation messages are 1KB-contiguous and each table is written
    with ONE fat DMA.  Both layers share one gather-index table
    (idx = col_local*4 + rel), resident in SBUF.
  - Gather slots use grouped continuous packing: per group of G dest tiles,
    each core packs its edges continuously into shared 128-slot blocks; the
    static schedule is a list of (block, tile) chunks whose one-hot S is
    built per chunk from a chunk-indexed dloc column (sentinel -1 for
    foreign/dummy slots).  This cuts dummy-slot gather waste from ~50% to
    ~6%.
  - Layer-1 aggregation is TRANSPOSED on TensorE (lhsT = gathered messages,
    rhs = S) so partials come out [H, node]; each dest tile is then
    immediately multiplied by W2cat on-core ((D t1)@W2 = D (t1@W2)), so the
    ReduceScatter moves 64-wide y2 partials and the layer-2 dense phase
    disappears.  dinv^2 scaling + b1-path bias are applied post-RS while
    assembling the layer-2 table.
  - Layer-2 aggregation is node-major (lhsT = S), 16-wide; after a second
    ReduceScatter a fused log_softmax (2 activation table loads total)
    emits a transposed [128, MC*16] output that the host un-transposes.
  Host work: index prep, degree/dinv, pre-scaling x by dinv, bf16 packing.
"""
import math
import os
import numpy as np
import ml_dtypes

import concourse.bass as bass
import concourse.tile as tile
from concourse import bacc, mybir
from concourse.bass_utils import run_bass_kernel_spmd

F32 = mybir.dt.float32
BF16 = mybir.dt.bfloat16
I16 = mybir.dt.int16
AF = mybir.ActivationFunctionType
ALU = mybir.AluOpType
BF_NP = ml_dtypes.bfloat16


class Cfg:
    def __init__(self, N, E, F, H, C, R, ncores=8, B=8, G=8, J=8):
        self.N, self.E, self.F, self.H, self.C, self.R = N, E, F, H, C, R
        self.ncores = ncores
        self.P = 128
        # node slice per core, multiple of 128 so dest tiles align to cores
        self.NLOC = math.ceil(N / ncores / 128) * 128
        self.MC = self.NLOC // 128                   # dest tiles per core
        self.MPAD = self.NLOC                        # padded nodes per core
        self.N_PAD = self.NLOC * ncores
        self.NT = self.N_PAD // 128                  # total dest tiles
        self.DUMMY = self.MPAD * R                   # zero row in tables
        self.B = B                                   # blocks per gather batch
        self.G = G                                   # tiles per packing group
        self.KC = F // 128                           # k-chunks layer-1 dense
        assert F % 128 == 0 and H == 128
        assert self.MPAD * R + 128 < 32768, "int16 gather index overflow"


CFG = Cfg(N=50000, E=800000, F=256, H=128, C=16, R=4,
          B=int(os.environ.get("KB", "8")), G=int(os.environ.get("KG", "8")))


def _bf(a):
    return np.ascontiguousarray(np.asarray(a, dtype=np.float32).astype(BF_NP))


# ----------------------------------------------------------------- host side
def preprocess(cfg, x, edge_index, edge_relation, W1, b1, W2, b2):
    N, ncores, NLOC, NT, R, MC, G = (cfg.N, cfg.ncores, cfg.NLOC, cfg.NT,
                                     cfg.R, cfg.MC, cfg.G)
    row = np.asarray(edge_index[0], dtype=np.int64)
    col = np.asarray(edge_index[1], dtype=np.int64)
    rel = np.asarray(edge_relation, dtype=np.int64)
    x = np.asarray(x, dtype=np.float32)
    W1 = np.asarray(W1, dtype=np.float32)
    b1 = np.asarray(b1, dtype=np.float32)
    W2 = np.asarray(W2, dtype=np.float32)
    b2 = np.asarray(b2, dtype=np.float32)

    deg = np.bincount(row, minlength=N).astype(np.float32)
    dinv = np.where(deg > 0, 1.0 / np.sqrt(np.maximum(deg, 1.0)),
                    0.0).astype(np.float32)

    # per-core edge sets (by source/col ownership), sorted by dest row
    per_core = []
    counts = np.zeros((ncores, NT), dtype=np.int64)
    for k in range(ncores):
        m = (col // NLOC) == k
        er, ec, eg = row[m], col[m] - k * NLOC, rel[m]
        o = np.argsort(er, kind="stable")
        er, ec, eg = er[o], ec[o], eg[o]
        counts[k] = np.bincount(er // 128, minlength=NT)
        per_core.append((er, ec, eg))

    # ---- grouped continuous packing (shared static schedule)
    # groups of up to G tiles, never crossing a dest-core boundary
    groups = []
    for dk in range(ncores):
        t0 = dk * MC
        for g0 in range(0, MC, G):
            groups.append((t0 + g0, t0 + min(MC, g0 + G)))
    gi_of_tile = np.zeros(NT, dtype=np.int64)
    for gi, (tlo, thi) in enumerate(groups):
        gi_of_tile[tlo:thi] = gi
    csum = [np.concatenate([[0], np.cumsum(counts[k])]) for k in range(ncores)]
    tiles_chunks = [[] for _ in range(NT)]   # per tile: list of block ids
    SB = 0
    core_gbase = np.zeros((ncores, len(groups)), dtype=np.int64)
    for gi, (tlo, thi) in enumerate(groups):
        nblk = 1
        for k in range(ncores):
            core_gbase[k, gi] = SB * 128
            sgk = int(csum[k][thi] - csum[k][tlo])
            nblk = max(nblk, math.ceil(sgk / 128))
        for t in range(tlo, thi):
            lo, hi = None, None
            for k in range(ncores):
                p0 = int(csum[k][t] - csum[k][tlo])
                p1 = int(csum[k][t + 1] - csum[k][tlo])
                if p1 > p0:
                    l, h = p0 // 128, (p1 - 1) // 128
                    lo = l if lo is None else min(lo, l)
                    hi = h if hi is None else max(hi, h)
            if lo is None:
                lo = hi = 0
            tiles_chunks[t] = [SB + b for b in range(lo, hi + 1)]
        SB += nblk
    NBAT = math.ceil(SB / cfg.B)
    SBpad = NBAT * cfg.B
    # chunk order = tile-major; chunk index per (tile, block)
    chunk_of = {}
    CH = 0
    for t in range(NT):
        for b in tiles_chunks[t]:
            chunk_of[(t, b)] = CH
            CH += 1

    # shared weight packs
    w1cat = np.zeros((cfg.KC, 128, R * cfg.H), dtype=np.float32)
    for r in range(R):
        for kc in range(cfg.KC):
            w1cat[kc, :, r * cfg.H:(r + 1) * cfg.H] = \
                W1[r * cfg.F + kc * 128: r * cfg.F + (kc + 1) * 128, :]
    w1cat = _bf(w1cat.reshape(cfg.KC * 128, R * cfg.H))
    w2cat = np.zeros((cfg.H, R * cfg.C), dtype=np.float32)
    for r in range(R):
        w2cat[:, r * cfg.C:(r + 1) * cfg.C] = W2[r * cfg.H:(r + 1) * cfg.H, :]
    w2cat = _bf(w2cat)
    crow1 = np.concatenate([b1 @ W2[r * cfg.H:(r + 1) * cfg.H, :]
                            for r in range(R)])
    crow = np.broadcast_to(crow1.astype(np.float32), (128, R * cfg.C)).copy()
    iota2 = _bf(np.broadcast_to(np.arange(128, dtype=np.float32), (128, 128)))
    b2r = np.broadcast_to(b2, (128, cfg.C)).astype(np.float32).copy()

    in_maps = []
    for k in range(ncores):
        er, ec, eg = per_core[k]
        t = er // 128
        egi = gi_of_tile[t]
        # position within group = running index of edge within its group
        first_of_group = np.concatenate(
            [[0], np.cumsum(np.bincount(egi, minlength=len(groups)))])[:-1]
        pos_in_group = np.arange(len(er)) - first_of_group[egi]
        slots = core_gbase[k][egi] + pos_in_group

        gidx = np.full(SBpad * 128, cfg.DUMMY, dtype=np.int16)
        gidx[slots] = (ec * R + eg).astype(np.int16)
        # chunk-indexed dloc: for chunk (t, b): slot p of block b ->
        #   er%128 if the edge belongs to tile t else -1
        dloc = np.full((CH, 128), -1.0, dtype=np.float32)
        eb = slots // 128
        ep = slots % 128
        eci = np.fromiter((chunk_of[(int(tt), int(bb))]
                           for tt, bb in zip(t, eb)),
                          dtype=np.int64, count=len(er))
        dloc[eci, ep] = (er % 128).astype(np.float32)

        # wrapped-16 int16 index layout, replicated to 8 Q7 groups, blocked
        # into one [128, NBAT*B*8] resident SBUF tile
        gg = gidx.reshape(NBAT, cfg.B * 8, 16)
        w = np.transpose(gg, (0, 2, 1))
        gidx_w = np.broadcast_to(
            w[:, None, :, :], (NBAT, 8, 16, cfg.B * 8)
        ).reshape(NBAT, 128, cfg.B * 8)
        gidx_sb = np.ascontiguousarray(
            np.transpose(gidx_w, (1, 0, 2)).reshape(128, NBAT * cfg.B * 8))
        dloc_sb = np.ascontiguousarray(dloc.T)       # [128, CH]

        lo = k * NLOC
        hi = min(N, lo + NLOC)
        nk = hi - lo
        uk = np.zeros((cfg.MPAD, cfg.F), dtype=np.float32)
        uk[:nk] = x[lo:hi] * dinv[lo:hi, None]
        dk = np.zeros(cfg.MPAD, dtype=np.float32)
        dk[:nk] = dinv[lo:hi]

        RC = R * cfg.C
        dinvcr = np.ascontiguousarray(
            np.repeat(dk.reshape(MC, 128), cfg.C, axis=1)
            .reshape(MC, 128, cfg.C).transpose(1, 0, 2)
            .reshape(128, MC * cfg.C))                            # [128, MC*C]
        dinvrc = np.ascontiguousarray(
            np.repeat(dk.reshape(MC, 128), RC, axis=1)
            .reshape(MC, 128, RC).transpose(1, 0, 2)
            .reshape(128, MC * RC))                               # [128, MC*RC]

        in_maps.append({
            "uT": _bf(uk.T),                     # [F, MPAD]
            "w1cat": w1cat,                      # [KC*128, R*H]
            "w2cat": w2cat,                      # [H, R*C]
            "crow": crow,                        # [128, R*C] f32
            "iota2": iota2,                      # [128, 128]
            "b2r": b2r,                          # [128, C] f32
            "gidx": gidx_sb,                     # [128, NBAT*B*8] i16
            "dloc": dloc_sb,                     # [128, CH] f32
            "dinvrc": dinvrc.astype(np.float32),  # [128, MC*RC] f32
            "dinvcr": dinvcr.astype(np.float32),  # [128, MC*C] f32
        })
    sched = tuple(tuple(tc_) for tc_ in tiles_chunks)
    return in_maps, sched, SBpad


# --------------------------------------------------------------- device side
def build_program(cfg, sched, SBpad):
    R, H, C = cfg.R, cfg.H, cfg.C
    NBAT = SBpad // cfg.B
    CH = sum(len(s) for s in sched)
    nc = bacc.Bacc("TRN2", target_bir_lowering=False, debug=False,
                   num_devices=cfg.ncores)

    uT = nc.dram_tensor("uT", [cfg.F, cfg.MPAD], BF16, kind="ExternalInput").ap()
    w1cat = nc.dram_tensor("w1cat", [cfg.KC * 128, R * H], BF16,
                           kind="ExternalInput").ap()
    w2cat = nc.dram_tensor("w2cat", [H, R * C], BF16, kind="ExternalInput").ap()
    crow = nc.dram_tensor("crow", [128, R * C], F32, kind="ExternalInput").ap()
    iota2 = nc.dram_tensor("iota2", [128, 128], BF16, kind="ExternalInput").ap()
    b2r = nc.dram_tensor("b2r", [128, C], F32, kind="ExternalInput").ap()
    gidx = nc.dram_tensor("gidx", [128, NBAT * cfg.B * 8], I16,
                          kind="ExternalInput").ap()
    dloc = nc.dram_tensor("dloc", [128, CH], F32, kind="ExternalInput").ap()
    dinvrc = nc.dram_tensor("dinvrc", [128, cfg.MC * R * C], F32,
                            kind="ExternalInput").ap()
    dinvcr = nc.dram_tensor("dinvcr", [128, cfg.MC * C], F32,
                            kind="ExternalInput").ap()
    outT = nc.dram_tensor("outT", [128, cfg.MC * C], F32,
                          kind="ExternalOutput").ap()

    with tile.TileContext(nc) as tc:
        _build(tc, cfg, sched, SBpad, uT, w1cat, w2cat, crow, iota2, b2r,
               gidx, dloc, dinvrc, dinvcr, outT)
    nc.compile()
    return nc


def _build(tc, cfg, sched, SBpad, uT, w1cat, w2cat, crow, iota2, b2r,
           gidx, dloc, dinvrc, dinvcr, outT):
    nc = tc.nc
    R, H, C, B, MC, NT, KC = (cfg.R, cfg.H, cfg.C, cfg.B, cfg.MC, cfg.NT,
                              cfg.KC)
    NBAT = SBpad // B
    NCORES = cfg.ncores
    TROWS = cfg.MPAD * R
    CH = sum(len(s) for s in sched)
    RC = R * C
    with tc.tile_pool(name="const", bufs=1) as cpool, \
         tc.tile_pool(name="big", bufs=1) as bigp, \
         tc.tile_pool(name="stg", bufs=2) as stgp, \
         tc.tile_pool(name="gY", bufs=6) as gpool, \
         tc.tile_pool(name="s3", bufs=12) as spool, \
         tc.tile_pool(name="ev", bufs=4) as evp, \
         tc.tile_pool(name="psA", bufs=2, space="PSUM") as ppa, \
         tc.tile_pool(name="psB", bufs=6, space="PSUM") as ppb, \
         tc.tile_pool(name="dram", bufs=1, space="DRAM") as dram:

        # ---------- resident inputs (dense-phase operands first)
        uT_t = bigp.tile([128, KC, cfg.MPAD], BF16)
        nc.sync.dma_start(
            out=uT_t[:],
            in_=uT.rearrange("(kc p) n -> p kc n", kc=KC, p=128))
        w1_t = cpool.tile([128, KC, R * H], BF16)
        nc.sync.dma_start(
            out=w1_t[:],
            in_=w1cat.rearrange("(kc p) n -> p kc n", kc=KC, p=128))
        w2_t = cpool.tile([128, RC], BF16)
        nc.sync.dma_start(out=w2_t[:], in_=w2cat[:, :])
        iota_t = cpool.tile([128, 128], BF16)
        nc.sync.dma_start(out=iota_t[:], in_=iota2[:, :])
        gidx_t = bigp.tile([128, NBAT * B * 8], I16)
        nc.sync.dma_start(out=gidx_t[:], in_=gidx[:, :])
        dloc_t = bigp.tile([128, CH], F32)
        nc.sync.dma_start(out=dloc_t[:], in_=dloc[:, :])
        crow_t = cpool.tile([128, RC], F32)
        nc.sync.dma_start(out=crow_t[:], in_=crow[:, :])
        b2_t = cpool.tile([128, C], F32)
        nc.sync.dma_start(out=b2_t[:], in_=b2r[:, :])
        dinvrc_t = bigp.tile([128, MC * RC], F32)
        nc.sync.dma_start(out=dinvrc_t[:], in_=dinvrc[:, :])
        dinvcr_t = bigp.tile([128, MC * C], F32)
        nc.sync.dma_start(out=dinvcr_t[:], in_=dinvcr[:, :])
        zrow = cpool.tile([128, 128], BF16)
        nc.vector.memset(zrow[:], 0.0)

        # shared table staging [128, MC*R*H] (y1: all cols; y2: :C per block)
        stage = bigp.tile([128, MC * R * H], BF16)

        # DRAM tensors
        y1_dram = dram.tile([TROWS + 128, H], BF16)
        y2_dram = dram.tile([TROWS + 128, H], BF16)
        y2_part = dram.tile([NCORES * 128, MC * RC], BF16)
        y2_red = dram.tile([128, MC * RC], BF16)
        t2_part = dram.tile([NCORES * 128, MC * C], BF16)
        t2_red = dram.tile([128, MC * C], BF16)

        # ---------- layer-1 dense: stage[p, mc*512 + r*H + h]
        for mc in range(MC):
            ps = ppa.tile([128, R * H], F32, tag="psA")
            for kc in range(KC):
                nc.tensor.matmul(
                    out=ps[:],
                    lhsT=uT_t[:, kc, mc * 128:(mc + 1) * 128],
                    rhs=w1_t[:, kc, :],
                    start=(kc == 0), stop=(kc == KC - 1))
            if mc % 2 == 0:
                nc.scalar.copy(out=stage[:, mc * R * H:(mc + 1) * R * H],
                               in_=ps[:])
            else:
                nc.vector.tensor_scalar(
                    out=stage[:, mc * R * H:(mc + 1) * R * H],
                    in0=ps[:], scalar1=1.0, scalar2=None, op0=ALU.mult)
        for q0 in range(0, MC, (MC + 3) // 4):
            q1 = min(MC, q0 + (MC + 3) // 4)
            nc.sync.dma_start(
                out=y1_dram[q0 * 512:q1 * 512, :].rearrange(
                    "(mc p r) h -> p mc (r h)", mc=q1 - q0, p=128, r=R),
                in_=stage[:, q0 * R * H:q1 * R * H])
        nc.sync.dma_start(out=y1_dram[TROWS:TROWS + 128, :], in_=zrow[:])

        LIMIT = int(os.environ.get("KLIMIT", "6"))
        if LIMIT < 2:
            return

        # ---------- shared gather + one-hot segment-sum pass
        def agg_pass(table_ap, width, part_dram, fuse_w2, tagsfx):
            batches = {}

            def batch(b):
                if b not in batches:
                    g = gpool.tile([128, B, H], BF16, tag="g")
                    nc.gpsimd.dma_gather(
                        out_ap=g[:], in_ap=table_ap,
                        idxs_ap=gidx_t[:, b * B * 8:(b + 1) * B * 8],
                        num_idxs=B * 128, num_idxs_reg=B * 128,
                        elem_size=H)
                    batches[b] = g
                return batches[b]

            TB = 4
            w0 = 128 if fuse_w2 else width
            ci = 0
            st = None
            ps = None
            for t in range(NT):
                k, j = t // MC, t % MC
                if j == 0:
                    st = stgp.tile([128, MC * width], BF16, tag=f"st{tagsfx}")
                jg = j % TB
                if jg == 0:
                    ntb = min(TB, MC - j)
                    ps = ppb.tile([128, TB * w0], F32, tag="ps")
                blocks = sched[t]
                for jj, b in enumerate(blocks):
                    g = batch(b // B)
                    s3 = spool.tile([128, 128], BF16, tag="s3")
                    nc.vector.tensor_scalar(
                        out=s3[:], in0=iota_t[:],
                        scalar1=dloc_t[:, ci:ci + 1],
                        scalar2=None, op0=ALU.is_equal)
                    if fuse_w2:
                        nc.tensor.matmul(
                            out=ps[:, jg * w0:(jg + 1) * w0],
                            lhsT=g[:, b % B, :], rhs=s3[:],
                            start=(jj == 0), stop=(jj == len(blocks) - 1))
                    else:
                        nc.tensor.matmul(
                            out=ps[:, jg * w0:(jg + 1) * w0],
                            lhsT=s3[:], rhs=g[:, b % B, :width],
                            start=(jj == 0), stop=(jj == len(blocks) - 1))
                    ci += 1
                if jg == ntb - 1:
                    j0 = j - ntb + 1
                    if fuse_w2:
                        # t1T tiles [h, node] -> (t1 @ W2cat) [node, RC]
                        tb = evp.tile([128, TB * 128], BF16, tag="t1T")
                        nc.scalar.copy(out=tb[:, :ntb * 128],
                                       in_=ps[:, :ntb * 128])
                        ps2 = ppa.tile([128, TB * RC], F32, tag="psA")
                        for q in range(ntb):
                            nc.tensor.matmul(
                                out=ps2[:, q * RC:(q + 1) * RC],
                                lhsT=tb[:, q * 128:(q + 1) * 128],
                                rhs=w2_t[:], start=True, stop=True)
                        nc.scalar.copy(
                            out=st[:, j0 * width:(j + 1) * width],
                            in_=ps2[:, :ntb * RC])
                    else:
                        nc.scalar.copy(
                            out=st[:, j0 * width:(j + 1) * width],
                            in_=ps[:, :ntb * width])
                if j == MC - 1:
                    nc.sync.dma_start(
                        out=part_dram[k * 128:(k + 1) * 128, :], in_=st[:])

        # ---------- layer-1 aggregation fused with W2 + reduce-scatter
        agg_pass(y1_dram[:, :], RC, y2_part, True, "1")
        if LIMIT < 3:
            return
        nc.gpsimd.collective_compute(
            "ReduceScatter", ALU.add,
            replica_groups=[list(range(NCORES))],
            ins=[y2_part.opt()], outs=[y2_red.opt()])
        if LIMIT < 4:
            return

        # ---------- layer-2 table: y2 = dinv * (dinv * red + crow)
        y2r_t = bigp.tile([128, MC * RC], BF16)
        nc.sync.dma_start(out=y2r_t[:], in_=y2_red[:, :])
        scal = bigp.tile([128, MC * RC], F32)
        nc.vector.tensor_tensor(out=scal[:], in0=y2r_t[:], in1=dinvrc_t[:],
                                op=ALU.mult)
        nc.vector.tensor_tensor(
            out=scal[:].rearrange("p (m rc) -> p m rc", rc=RC),
            in0=scal[:].rearrange("p (m rc) -> p m rc", rc=RC),
            in1=crow_t[:].unsqueeze(1).to_broadcast([128, MC, RC]),
            op=ALU.add)
        nc.vector.tensor_tensor(out=scal[:], in0=scal[:], in1=dinvrc_t[:],
                                op=ALU.mult)
        nc.vector.tensor_scalar(
            out=stage[:].rearrange("p (m r h) -> p m r h", r=R, h=H)
                [:, :, :, 0:C],
            in0=scal[:].rearrange("p (m r c) -> p m r c", r=R, c=C),
            scalar1=1.0, scalar2=None, op0=ALU.mult)
        for q0 in range(0, MC, (MC + 3) // 4):
            q1 = min(MC, q0 + (MC + 3) // 4)
            nc.sync.dma_start(
                out=y2_dram[q0 * 512:q1 * 512, :].rearrange(
                    "(mc p r) h -> p mc (r h)", mc=q1 - q0, p=128, r=R),
                in_=stage[:, q0 * R * H:q1 * R * H])
        nc.sync.dma_start(out=y2_dram[TROWS:TROWS + 128, :], in_=zrow[:])
        if LIMIT < 5:
            return

        # ---------- layer-2 aggregation (node-major) + reduce-scatter
        agg_pass(y2_dram[:, :], C, t2_part, False, "2")
        nc.gpsimd.collective_compute(
            "ReduceScatter", ALU.add,
            replica_groups=[list(range(NCORES))],
            ins=[t2_part.opt()], outs=[t2_red.opt()])
        if LIMIT < 6:
            return

        # ---------- final: h2 = dinv*t2 + b2 ; log_softmax over C
        t2r_t = bigp.tile([128, MC * C], BF16)
        nc.sync.dma_start(out=t2r_t[:], in_=t2_red[:, :])
        h2 = bigp.tile([128, MC, C], F32)
        nc.vector.tensor_tensor(
            out=h2[:], in0=t2r_t[:].rearrange("p (m c) -> p m c", c=C),
            in1=dinvcr_t[:].rearrange("p (m c) -> p m c", c=C), op=ALU.mult)
        nc.vector.tensor_tensor(
            out=h2[:], in0=h2[:],
            in1=b2_t[:].unsqueeze(1).to_broadcast([128, MC, C]), op=ALU.add)
        negmx = bigp.tile([128, MC, 1], F32)
        nc.vector.tensor_reduce(out=negmx[:], in_=h2[:],
                                axis=mybir.AxisListType.X,
                                op=ALU.max, negate=True)
        nc.vector.tensor_tensor(
            out=h2[:], in0=h2[:], in1=negmx[:].to_broadcast([128, MC, C]),
            op=ALU.add)
        ex = bigp.tile([128, MC, C], F32)
        nc.scalar.activation(out=ex[:], in_=h2[:], func=AF.Exp)
        ssum = bigp.tile([128, MC, 1], F32)
        nc.vector.tensor_reduce(out=ssum[:], in_=ex[:],
                                axis=mybir.AxisListType.X, op=ALU.add)
        lg = bigp.tile([128, MC, 1], F32)
        nc.scalar.activation(out=lg[:], in_=ssum[:], func=AF.Ln)
        nc.vector.tensor_tensor(
            out=h2[:], in0=h2[:], in1=lg[:].to_broadcast([128, MC, C]),
            op=ALU.subtract)
        nc.sync.dma_start(
            out=outT[:, :], in_=h2[:].rearrange("p m c -> p (m c)"))


# ------------------------------------------------------------------ runtime
_PROGRAM_CACHE = {}


def run(cfg, inputs):
    in_maps, sched, SBpad = preprocess(cfg, **inputs)
    key = (cfg.N, cfg.E, sched, SBpad)
    if key not in _PROGRAM_CACHE:
        _PROGRAM_CACHE[key] = build_program(cfg, sched, SBpad)
    nc = _PROGRAM_CACHE[key]
    res = None
    for attempt in range(3):
        try:
            res = run_bass_kernel_spmd(nc, in_maps,
                                       core_ids=list(range(cfg.ncores)))
            break
        except Exception:
            if attempt == 2:
                raise
    outs = []
    for k in range(cfg.ncores):
        oT = res.results[k]["outT"]                       # [128, MC*C]
        o = oT.reshape(128, cfg.MC, cfg.C).transpose(1, 0, 2).reshape(
            cfg.NLOC, cfg.C)
        outs.append(o)
    full = np.concatenate(outs, axis=0)[:cfg.N]
    return np.ascontiguousarray(full.astype(np.float32))


def kernel(x, edge_index, edge_relation, W1, b1, W2, b2):
    return run(CFG, dict(x=x, edge_index=edge_index,
                         edge_relation=edge_relation,
                         W1=W1, b1=b1, W2=W2, b2=b2))
